# revision 6
# baseline (speedup 1.0000x reference)
"""GCN2 (gnn_message_passing) Trainium2 kernel over 8 NeuronCores.

Strategy (graph/data parallel, dst-sharded):
  - Nodes are sharded by range across the 8 cores (12500 each, padded to
    12544 = 98 blocks of 128).  Each core owns the edges whose *destination*
    falls in its range.
  - Between layers the scaled feature table g = dinv * h (bf16) is
    AllGather'ed so every core holds the full [100352, 256] table in HBM.
  - Per 128-dst-node block: bulk-gather the source rows with dma_gather
    (int16 indices -> 4 windows of 25088 rows), then segment-sum via one-hot
    matmuls on the PE.  The one-hot selection matrix S[e, d] = (dst[e] == d)
    is generated on-device by a single DVE is_equal against an iota row tile,
    so no S traffic from HBM.  norm = dinv[src]*dinv[dst] is realized by
    pre-scaling rows by dinv (src side) and a per-partition dinv scale fused
    into the epilogue (dst side), keeping S exactly 0/1.
  - Epilogue per block: z = (1-a)*agg + a*x0 (fused scalar_tensor_tensor),
    zs = (1-beta)z (bf16), PE-transpose zs, hp = zs @ ((beta/(1-beta))W),
    h = relu(zs + hp); g_next = dinv*h.  Final layer computes the
    classification head (logits, softmax, argmax) on-device.
"""

import math
from contextlib import ExitStack

import ml_dtypes
import numpy as np

import concourse.bass as bass
import concourse.mybir as mybir
import concourse.tile as tile
from concourse import bacc
from concourse.bass import ds
from concourse.bass_utils import run_bass_kernel_spmd
from concourse.masks import make_identity

# -------------------- problem constants (hardcoded) --------------------
N = 100000
FIN, H, FOUT, L = 512, 256, 64, 8
ALPHA, THETA = 0.1, 0.5
NCORES = 8
SHARD = N // NCORES          # 12500
P = 128
BLOCKS = (SHARD + P - 1) // P  # 98
NPAD = BLOCKS * P            # 12544
NTOT = NCORES * NPAD         # 100352
WIN = 4                      # index windows (int16 limit of dma_gather)
WROWS = NTOT // WIN          # 25088

BF16 = mybir.dt.bfloat16
F32 = mybir.dt.float32
I16 = mybir.dt.int16
U32 = mybir.dt.uint32

BETAS = [float(np.log(THETA / (i + 1) + 1.0)) for i in range(L)]


# -------------------- host-side preprocessing --------------------
def _prep(x, edge_index, W0, b0, Ws, Wc, bc):
    src = np.asarray(edge_index[0], dtype=np.int64)
    dst = np.asarray(edge_index[1], dtype=np.int64)
    loops = np.arange(N, dtype=np.int64)
    src = np.concatenate([src, loops])
    dst = np.concatenate([dst, loops])

    deg = np.bincount(dst, minlength=N).astype(np.float32)
    dinv = (1.0 / np.sqrt(deg)).astype(np.float32)

    # gather position of src node in the AllGather'ed padded table
    pos = (src // SHARD) * NPAD + (src % SHARD)
    core = dst // SHARD
    ld = dst - core * SHARD
    blk = ld // P
    row = (ld % P).astype(np.int64)
    win = pos // WROWS
    wix = (pos % WROWS).astype(np.int64)
    grp = (core * BLOCKS + blk) * WIN + win

    order = np.lexsort((wix, grp))
    grp_s = grp[order]
    wix_s = wix[order]
    row_s = row[order]

    ngrp = NCORES * BLOCKS * WIN
    cnt = np.bincount(grp_s, minlength=ngrp)
    CW = max(P, int(math.ceil(cnt.max() / P)) * P)
    starts = np.zeros(ngrp, dtype=np.int64)
    np.cumsum(cnt[:-1], out=starts[1:])
    pig = np.arange(len(grp_s), dtype=np.int64) - starts[grp_s]

    gidx_f = np.zeros((ngrp, CW), np.int16)
    gidx_f[grp_s, pig] = wix_s.astype(np.int16)
    drow_f = np.full((ngrp, CW), 255, np.int32)
    drow_f[grp_s, pig] = row_s

    # dma_gather wrapped-16 index layout, windows concatenated along columns,
    # replicated to 128 partitions.
    gw = gidx_f.reshape(NCORES, BLOCKS, WIN, CW // 16, 16)
    gw = gw.transpose(0, 1, 4, 2, 3).reshape(NCORES, BLOCKS, 16, WIN * (CW // 16))
    gidx_in = np.ascontiguousarray(
        np.tile(gw, (1, 1, 8, 1)).reshape(NCORES, BLOCKS * P, WIN * (CW // 16))
    )

    # dst-row matrix [core, block*128(edge partition), chunk] (bf16, 255=pad)
    dm = drow_f.reshape(NCORES, BLOCKS, WIN, CW // P, P)
    dm = dm.transpose(0, 1, 4, 2, 3).reshape(NCORES, BLOCKS * P, WIN * (CW // P))
    dstm_in = np.ascontiguousarray(dm.astype(ml_dtypes.bfloat16))

    # per-node dinv columns: [0.9*dinv, dinv]
    dv = np.ones((NCORES, NPAD), np.float32)
    dv[:, :SHARD] = dinv.reshape(NCORES, SHARD)
    dinv2_in = np.ascontiguousarray(
        np.stack([(1.0 - ALPHA) * dv, dv], axis=-1).reshape(NCORES, NPAD, 2)
    )

    # x transposed per block: [core, block, kchunk, k, n]
    xp = np.zeros((NCORES, NPAD, FIN), np.float32)
    xp[:, :SHARD] = np.asarray(x, np.float32).reshape(NCORES, SHARD, FIN)
    xT = xp.reshape(NCORES, BLOCKS, P, FIN // P, P).transpose(0, 1, 3, 4, 2)
    xT_in = np.ascontiguousarray(xT.reshape(NCORES, BLOCKS * FIN, P))

    # weights (replicated)
    w0_in = np.ascontiguousarray(np.asarray(W0, np.float32).reshape(FIN, H))
    b0_in = np.asarray(b0, np.float32).reshape(1, H)
    wr = np.stack(
        [(BETAS[i] / (1.0 - BETAS[i])) * np.asarray(Ws[i], np.float32) for i in range(L)]
    )
    wr_in = np.ascontiguousarray(wr.reshape(L * 2 * P, H).astype(ml_dtypes.bfloat16))
    wc_in = np.ascontiguousarray(np.asarray(Wc, np.float32).reshape(2 * P, FOUT))
    bc_in = np.asarray(bc, np.float32).reshape(1, FOUT)
    ones_in = np.ones((1, P), np.float32)
    iota_in = np.ascontiguousarray(
        np.broadcast_to(np.arange(P, dtype=np.float32), (P, P)).astype(ml_dtypes.bfloat16)
    )

    in_maps = []
    for c in range(NCORES):
        in_maps.append(
            {
                "xT": xT_in[c],
                "gidx": gidx_in[c],
                "dstm": dstm_in[c],
                "dinv2": dinv2_in[c],
                "w0": w0_in,
                "b0r": b0_in,
                "wr": wr_in,
                "wc": wc_in,
                "bcr": bc_in,
                "ones1": ones_in,
                "iota": iota_in,
            }
        )
    return in_maps, CW


# -------------------- device program --------------------
def _build(CW):
    NCH = WIN * CW // P       # gather chunks per block
    IDXC = WIN * CW // 16     # int16 index columns per block
    CWP = CW // P             # chunks per window
    CW16 = CW // 16

    nc = bacc.Bacc("TRN2", target_bir_lowering=False, debug=False, num_devices=NCORES)

    t_xT = nc.dram_tensor("xT", [BLOCKS * FIN, P], F32, kind="ExternalInput")
    t_gidx = nc.dram_tensor("gidx", [BLOCKS * P, IDXC], I16, kind="ExternalInput")
    t_dstm = nc.dram_tensor("dstm", [BLOCKS * P, NCH], BF16, kind="ExternalInput")
    t_dinv = nc.dram_tensor("dinv2", [BLOCKS * P, 2], F32, kind="ExternalInput")
    t_w0 = nc.dram_tensor("w0", [FIN, H], F32, kind="ExternalInput")
    t_b0 = nc.dram_tensor("b0r", [1, H], F32, kind="ExternalInput")
    t_wr = nc.dram_tensor("wr", [L * 2 * P, H], BF16, kind="ExternalInput")
    t_wc = nc.dram_tensor("wc", [2 * P, FOUT], F32, kind="ExternalInput")
    t_bc = nc.dram_tensor("bcr", [1, FOUT], F32, kind="ExternalInput")
    t_ones = nc.dram_tensor("ones1", [1, P], F32, kind="ExternalInput")
    t_iota = nc.dram_tensor("iota", [P, P], BF16, kind="ExternalInput")

    o_emb = nc.dram_tensor("emb", [NPAD, H], F32, kind="ExternalOutput")
    o_log = nc.dram_tensor("logits", [NPAD, FOUT], F32, kind="ExternalOutput")
    o_soft = nc.dram_tensor("soft", [NPAD, FOUT], F32, kind="ExternalOutput")
    o_hard = nc.dram_tensor("hard", [NPAD, 1], U32, kind="ExternalOutput")

    RG = [list(range(NCORES))]
    Alu = mybir.AluOpType
    Act = mybir.ActivationFunctionType

    with tile.TileContext(nc) as tc, ExitStack() as ctx:
        const = ctx.enter_context(tc.tile_pool(name="const", bufs=1))
        sb = ctx.enter_context(tc.tile_pool(name="sb", bufs=3))
        gp = ctx.enter_context(tc.tile_pool(name="gp", bufs=2))
        pp = ctx.enter_context(tc.tile_pool(name="pp", bufs=2, space="PSUM"))
        dr1 = ctx.enter_context(tc.tile_pool(name="dr1", bufs=1, space="DRAM"))
        drs = ctx.enter_context(tc.tile_pool(name="drs", bufs=2, space="DRAM"))
        drf = ctx.enter_context(tc.tile_pool(name="drf", bufs=2, space="DRAM"))

        # ---- constants ----
        w0_t = const.tile([P, FIN // P, H], F32, name="w0_t")
        for k in range(FIN // P):
            nc.sync.dma_start(w0_t[:, k, :], t_w0[k * P : (k + 1) * P, :])
        b0_t = const.tile([1, H], F32, name="b0_t")
        nc.sync.dma_start(b0_t[:], t_b0[:, :])
        wr_t = const.tile([P, L * 2, H], BF16, name="wr_t")
        for k in range(L * 2):
            nc.sync.dma_start(wr_t[:, k, :], t_wr[k * P : (k + 1) * P, :])
        wc_t = const.tile([P, 2, FOUT], F32, name="wc_t")
        for k in range(2):
            nc.sync.dma_start(wc_t[:, k, :], t_wc[k * P : (k + 1) * P, :])
        bc_t = const.tile([1, FOUT], F32, name="bc_t")
        nc.sync.dma_start(bc_t[:], t_bc[:, :])
        ones_t = const.tile([1, P], F32, name="ones_t")
        nc.sync.dma_start(ones_t[:], t_ones[:, :])
        iota_t = const.tile([P, P], BF16, name="iota_t")
        nc.sync.dma_start(iota_t[:], t_iota[:, :])
        identb = const.tile([P, P], BF16, name="identb")
        make_identity(nc, identb[:])
        identf = const.tile([P, P], F32, name="identf")
        make_identity(nc, identf[:])

        x0a_d = dr1.tile([NPAD, H], F32, name="x0a_d")
        gsh = [drs.tile([NPAD, H], BF16, name=f"gsh{i}") for i in range(L)]
        gfull = [
            drf.tile([NTOT, H], BF16, addr_space="Shared", name=f"gfull{i}")
            for i in range(L)
        ]

        # ---- phase 0: x0 = relu(x @ W0 + b0); x0a = 0.1*x0; g0 = dinv*x0 ----
        def x0_body(i):
            xt = sb.tile([P, FIN // P, P], F32, tag="xt", name="xt")
            for k in range(FIN // P):
                nc.sync.dma_start(xt[:, k, :], t_xT[ds(i * FIN + k * P, P), :])
            dv = sb.tile([P, 2], F32, tag="dv", name="dv")
            nc.sync.dma_start(dv[:], t_dinv[ds(i * P, P), :])
            ps = pp.tile([P, H], F32, tag="agg", name="ps0")
            for k in range(FIN // P):
                nc.tensor.matmul(ps[:], lhsT=xt[:, k, :], rhs=w0_t[:, k, :],
                                 start=(k == 0), stop=False)
            nc.tensor.matmul(ps[:], lhsT=ones_t[:], rhs=b0_t[:], start=False, stop=True)
            x0a = sb.tile([P, H], F32, tag="x0a", name="x0a")
            nc.scalar.activation(x0a[:], ps[:], Act.Relu, scale=ALPHA)
            nc.sync.dma_start(x0a_d[ds(i * P, P), :], x0a[:])
            g0 = sb.tile([P, H], BF16, tag="gt", name="g0")
            nc.scalar.activation(g0[:], ps[:], Act.Relu, scale=dv[:, 1:2])
            nc.sync.dma_start(gsh[0][ds(i * P, P), :], g0[:])

        tc.For_i_unrolled(0, BLOCKS, 1, x0_body, max_unroll=7)

        nc.gpsimd.collective_compute(
            "AllGather", Alu.bypass, replica_groups=RG,
            ins=[gsh[0].opt()], outs=[gfull[0].opt()],
        )

        # ---- message-passing layers ----
        for layer in range(L):
            gin = gfull[layer]
            gout = gsh[layer] if layer < L - 1 else None
            one_minus_beta = 1.0 - BETAS[layer]
            last = layer == L - 1

            def body(i, layer=layer, gin=gin, gout=gout,
                     one_minus_beta=one_minus_beta, last=last):
                gix = sb.tile([P, IDXC], I16, tag="gix", name="gix")
                nc.sync.dma_start(gix[:], t_gidx[ds(i * P, P), :])
                dsm = sb.tile([P, NCH], BF16, tag="dsm", name="dsm")
                nc.sync.dma_start(dsm[:], t_dstm[ds(i * P, P), :])
                dv = sb.tile([P, 2], F32, tag="dv", name="dv")
                nc.sync.dma_start(dv[:], t_dinv[ds(i * P, P), :])
                x0a = sb.tile([P, H], F32, tag="x0a", name="x0a")
                nc.sync.dma_start(x0a[:], x0a_d[ds(i * P, P), :])

                G = gp.tile([P, NCH, H], BF16, tag="G", name="G")
                GMAX = 1024  # dma_gather faults above 1024 idxs per call
                for w in range(WIN):
                    for off in range(0, CW, GMAX):
                        n = min(GMAX, CW - off)
                        c0 = w * CWP + off // P
                        nc.gpsimd.dma_gather(
                            G[:, c0 : c0 + n // P, :],
                            gin[w * WROWS : (w + 1) * WROWS, :],
                            gix[:, w * CW16 + off // 16 : w * CW16 + (off + n) // 16],
                            n, n, H,
                        )

                S = sb.tile([P, NCH, P], BF16, tag="S", name="S")
                nc.vector.tensor_tensor(
                    out=S[:],
                    in0=dsm[:].unsqueeze(2).to_broadcast([P, NCH, P]),
                    in1=iota_t[:].unsqueeze(1).to_broadcast([P, NCH, P]),
                    op=Alu.is_equal,
                )

                ps = pp.tile([P, H], F32, tag="agg", name="agg")
                for ch in range(NCH):
                    nc.tensor.matmul(ps[:], lhsT=S[:, ch, :], rhs=G[:, ch, :],
                                     start=(ch == 0), stop=(ch == NCH - 1))

                z = sb.tile([P, H], F32, tag="z", name="z")
                nc.vector.scalar_tensor_tensor(
                    out=z[:], in0=ps[:], scalar=dv[:, 0:1], in1=x0a[:],
                    op0=Alu.mult, op1=Alu.add,
                )
                zs = sb.tile([P, H], BF16, tag="zs", name="zs")
                nc.scalar.activation(zs[:], z[:], Act.Copy, scale=one_minus_beta)

                zT = sb.tile([P, 2, P], BF16, tag="zT", name="zT")
                for k in range(2):
                    tp = pp.tile([P, P], BF16, tag="tp", name="tp")
                    nc.tensor.transpose(tp[:], zs[:, k * P : (k + 1) * P], identb[:])
                    nc.scalar.copy(zT[:, k, :], tp[:])

                hp = pp.tile([P, H], F32, tag="hp", name="hp")
                for k in range(2):
                    nc.tensor.matmul(hp[:], lhsT=zT[:, k, :],
                                     rhs=wr_t[:, layer * 2 + k, :],
                                     start=(k == 0), stop=(k == 1))

                t = sb.tile([P, H], F32, tag="tt", name="tt")
                nc.vector.tensor_tensor(out=t[:], in0=hp[:], in1=zs[:], op=Alu.add)

                if not last:
                    gt = sb.tile([P, H], BF16, tag="gt", name="gt")
                    nc.scalar.activation(gt[:], t[:], Act.Relu, scale=dv[:, 1:2])
                    nc.sync.dma_start(gout[ds(i * P, P), :], gt[:])
                else:
                    emb = sb.tile([P, H], F32, tag="emb", name="emb")
                    nc.scalar.activation(emb[:], t[:], Act.Relu)
                    nc.sync.dma_start(o_emb[ds(i * P, P), :], emb[:])
                    hT = sb.tile([P, 2, P], F32, tag="hT", name="hT")
                    for k in range(2):
                        tp2 = pp.tile([P, P], F32, tag="tp", name="tp2")
                        nc.tensor.transpose(tp2[:], emb[:, k * P : (k + 1) * P],
                                            identf[:])
                        nc.scalar.copy(hT[:, k, :], tp2[:])
                    lp = pp.tile([P, FOUT], F32, tag="lp", name="lp")
                    for k in range(2):
                        nc.tensor.matmul(lp[:], lhsT=hT[:, k, :], rhs=wc_t[:, k, :],
                                         start=(k == 0), stop=False)
                    nc.tensor.matmul(lp[:], lhsT=ones_t[:], rhs=bc_t[:],
                                     start=False, stop=True)
                    lg = sb.tile([P, FOUT], F32, tag="lg", name="lg")
                    nc.vector.tensor_copy(lg[:], lp[:])
                    nc.sync.dma_start(o_log[ds(i * P, P), :], lg[:])
                    # softmax
                    mx = sb.tile([P, 1], F32, tag="mx", name="mx")
                    nc.vector.tensor_reduce(mx[:], lg[:], axis=mybir.AxisListType.X,
                                            op=Alu.max, negate=True)
                    ex = sb.tile([P, FOUT], F32, tag="ex", name="ex")
                    nc.scalar.activation(ex[:], lg[:], Act.Exp, bias=mx[:, 0:1])
                    sm = sb.tile([P, 1], F32, tag="sm", name="sm")
                    nc.vector.tensor_reduce(sm[:], ex[:], axis=mybir.AxisListType.X,
                                            op=Alu.add)
                    rc = sb.tile([P, 1], F32, tag="rc", name="rc")
                    nc.vector.reciprocal(rc[:], sm[:])
                    sf = sb.tile([P, FOUT], F32, tag="sf", name="sf")
                    nc.vector.tensor_scalar_mul(sf[:], ex[:], rc[:, 0:1])
                    nc.sync.dma_start(o_soft[ds(i * P, P), :], sf[:])
                    # argmax
                    m8 = sb.tile([P, 8], F32, tag="m8", name="m8")
                    nc.vector.max(out=m8[:], in_=lg[:])
                    i8 = sb.tile([P, 8], U32, tag="i8", name="i8")
                    nc.vector.max_index(out=i8[:], in_max=m8[:], in_values=lg[:])
                    nc.sync.dma_start(o_hard[ds(i * P, P), :], i8[:, 0:1])

            tc.For_i_unrolled(0, BLOCKS, 1, body, max_unroll=7)

            if not last:
                nc.gpsimd.collective_compute(
                    "AllGather", Alu.bypass, replica_groups=RG,
                    ins=[gout.opt()], outs=[gfull[layer + 1].opt()],
                )

    nc.compile()
    return nc


# -------------------- entry point --------------------
def kernel(x, edge_index, W0, b0, Ws, Wc, bc, _trace=False, _tmpdir=None):
    in_maps, CW = _prep(x, edge_index, W0, b0, Ws, Wc, bc)
    nc = _build(CW)
    res = run_bass_kernel_spmd(
        nc, in_maps, core_ids=list(range(NCORES)), trace=_trace, tmpdir=_tmpdir
    )
    outs = res.results

    def gather(name, width):
        parts = [outs[c][name][:SHARD] for c in range(NCORES)]
        return np.concatenate(parts, axis=0)

    logits = gather("logits", FOUT).astype(np.float32)
    embedding = gather("emb", H).astype(np.float32)
    soft = gather("soft", FOUT).astype(np.float32)
    hard = gather("hard", 1).reshape(-1).astype(np.int32)
    kernel._last_result = res
    return logits, embedding, soft, hard
